# revision 1
# baseline (speedup 1.0000x reference)
"""Trainium2 Bass kernel for nn_AttentionMoeModel — v3 (f32r pre-routing).

Sharding as baseline. Speed comes from:
  - float32r matmuls (1 cy/row at >=256 moving dim) for everything upstream of
    the MoE routers — same precision as the HW fp32 path (~1.4e-4), which the
    razor-thin top-2 routing margins require — fp16 only post-routing
    (layer-3 FFN products + lm_head).
  - weights host-prepacked for few big DMAs; fc/proj/w1/w2 streamed per-block
  - scores/exp/den/pv interleaved per (ch, j): p never materialized fully
  - softmax 1/den folded into o-proj PSUM copy; rq broadcast via one-hot
    matmuls; wf AllGather overlapped with shared-expert/w1 compute
"""
import sys

sys.path.insert(0, "/opt/trn_rl_repo")

from contextlib import ExitStack

import numpy as np

import concourse.bass as bass
import concourse.mybir as mybir
import concourse.tile as tile
from concourse import bacc
from concourse.bass import IndirectOffsetOnAxis
from concourse.bass_utils import run_bass_kernel_spmd
from concourse.masks import make_identity

B, T, D, H, HD, V, L = 1, 1024, 1024, 8, 128, 32000, 4
E, F = 8, 1024
DENSE_N = 2
VE_LAYERS = {0: 0, 3: 1}
WINDOWS = [1024, 512, 1024, 1024]
VE_GATE_CH = 32

NCORE = 8
P = 128
TS = T // NCORE
ND = D // P
NT = T // P
VS = V // NCORE
CH = 512
NCH = T // CH
NF = 4
EPS = 1e-6

f32 = mybir.dt.float32
f16 = mybir.dt.float16
fr = mybir.dt.float32r
i32 = mybir.dt.int32
AF = mybir.ActivationFunctionType
OP = mybir.AluOpType
AX = mybir.AxisListType
NPH = np.float16


def _rope_tables():
    inv = 1.0 / (10000.0 ** (np.arange(0, HD, 2, dtype=np.float64) / HD))
    fri = np.arange(T, dtype=np.float64)[:, None] * inv[None, :]
    cos, sin = np.cos(fri), np.sin(fri)
    cc = np.empty((P, T), np.float32)
    ss = np.empty((P, T), np.float32)
    cc[:64] = cos.T
    cc[64:] = cos.T
    ss[:64] = sin.T
    ss[64:] = -sin.T
    return cc, ss


def _block_mask(w, j, ch):
    tk = np.arange(P)[:, None] + P * j
    tq = np.arange(CH)[None, :] + CH * ch
    return ((tk <= tq) & (tq - tk <= w)).astype(np.float32)


def _mask_plan():
    uniq, keys, plan = [], {}, {}
    for w in set(WINDOWS):
        plan[w] = {}
        for j in range(NT):
            for ch in range(NCH):
                m = _block_mask(w, j, ch)
                if not m.any():
                    plan[w][(j, ch)] = "skip"
                elif m.all():
                    plan[w][(j, ch)] = "full"
                else:
                    kb = m.tobytes()
                    if kb not in keys:
                        keys[kb] = len(uniq)
                        uniq.append(m)
                    plan[w][(j, ch)] = keys[kb]
    return np.stack(uniq), plan


MASKS, MASK_PLAN = _mask_plan()
NMASK = MASKS.shape[0]


def _pack(w, blk):
    """[Kb*blk, C] -> [blk, Kb*C] with pack[p, kb*C+c] = w[kb*blk+p, c]."""
    kb = w.shape[0] // blk
    return np.ascontiguousarray(
        w.reshape(kb, blk, w.shape[1]).transpose(1, 0, 2).reshape(blk, kb * w.shape[1]))


def _pack_lmh(w):
    """[D, VS] -> [128, NVB*ND*128]: out[p, (vb*ND+db)*128+c] =
    w[db*128+p, vb*128+c], zero-padded in vb tail."""
    nvb = (VS + P - 1) // P
    out = np.zeros((P, nvb * ND * P), w.dtype)
    for vb in range(nvb):
        vm = min(P, VS - vb * P)
        blk = w[:, vb * P:vb * P + vm]            # [D, vm]
        r = blk.reshape(ND, P, vm)                # [db, p, c]
        for db in range(ND):
            out[:, (vb * ND + db) * P:(vb * ND + db) * P + vm] = r[db]
    return out


def _pack_fb(w, nfb):
    """[D, Fb*128] -> [128, nfb * (ND*128)]: out[p, fb*D + db*128 + c] =
    w[db*128+p, fb*128+c]  (per-fb stationary tiles for w1/fc)."""
    Din = w.shape[0]
    nd = Din // P
    r = w.reshape(nd, P, nfb, P).transpose(1, 2, 0, 3).reshape(P, nfb * nd * P)
    return np.ascontiguousarray(r)


class Builder:
    def __init__(self, nc, tc, ia):
        self.nc = nc
        self.tc = tc
        self.ia = ia
        self.uid = 0

    def name(self, s):
        self.uid += 1
        return f"{s}_{self.uid}"

    def dram(self, s, shape, dtype=f32, shared=False):
        if shared:
            return self.nc.dram_tensor(self.name(s), shape, dtype, addr_space="Shared")
        return self.nc.dram_tensor(self.name(s), shape, dtype)

    # ---- small helpers -----------------------------------------------------
    def cp(self, i, out, in_):
        if i % 2 == 0:
            self.nc.scalar.copy(out, in_)
        else:
            self.nc.vector.tensor_copy(out, in_)

    def rms_tm(self, out_pool, out_tag, x):
        nc = self.nc
        scr = self.wk1.tile([P, D], f16, name=self.name("rms_scr"), tag="sq")
        ssq = self.sm.tile([P, 1], f32, name=self.name("ssq"), tag="sm1")
        nc.scalar.activation(scr[:], x[:], AF.Square, accum_out=ssq[:, :1])
        s1 = self.sm.tile([P, 1], f32, name=self.name("rms_s1"), tag="sm1")
        nc.scalar.activation(s1[:], ssq[:], AF.Sqrt, bias=self.eps[:, :1], scale=1.0 / D)
        s2 = self.sm.tile([P, 1], f32, name=self.name("rms_s2"), tag="sm1")
        nc.vector.reciprocal(s2[:], s1[:])
        xn = out_pool.tile([P, D], f32, name=self.name("rms_out"), tag=out_tag)
        nc.scalar.mul(xn[:], x[:], s2[:, :1])
        return xn

    def row_to_tm(self, row):
        nc = self.nc
        db = self.dram("tb", [1, NT * P])
        nc.sync.dma_start(db.ap()[:], row[:])
        out = self.sm.tile([P, NT], f32, name=self.name("tmn"), tag="smn")
        nc.sync.dma_start(out[:], db.ap().rearrange("o (j p) -> (o p) j", p=P)[:])
        return out

    def allgather_fm(self, xn, nm, dt=fr, want_f32T=False):
        """xn [128, D] tm fp32 -> x_fm [128, ND, T] (dt) in 'big' pool."""
        nc = self.nc
        xnT = self.wk1.tile([P, ND, TS], dt, name=self.name("xnT"), tag="xnT")
        xnT32 = None
        if want_f32T:
            xnT32 = self.wk1.tile([P, ND, TS], f32, name=self.name("xnT32"),
                                  tag="xnT32")
        for db in range(ND):
            pt = self.ps.tile([P, CH], f32, name=self.name("ps_tr"), tag="ps")
            nc.tensor.transpose(pt[:, :P], xn[:, db * P:(db + 1) * P], self.idn[:])
            self.cp(db, xnT[:, db, :], pt[:, :P])
            if want_f32T:
                nc.vector.tensor_copy(xnT32[:, db, :], pt[:, :P])
        agin = self.dram("agin", [D, TS], dt)
        nc.sync.dma_start(agin.ap().rearrange("(n p) t -> p n t", p=P)[:], xnT[:])
        agout = self.dram("agout", [NCORE * D, TS], dt, shared=True)
        nc.gpsimd.collective_compute(
            "AllGather", OP.bypass, replica_groups=[list(range(NCORE))],
            ins=[agin.ap()[:]], outs=[agout.ap()[:]],
        )
        x_fm = self.big.tile([P, ND, T], dt, name=self.name(nm), tag="big")
        agv = agout.ap().rearrange("(r n p) t -> p n r t", p=P, n=ND)
        for db in range(ND):
            nc.sync.dma_start(
                x_fm.rearrange("p n (r t) -> p n r t", r=NCORE)[:, db, :, :],
                agv[:, db, :, :],
            )
        return x_fm, xnT, xnT32

    def reduce_scatter_add(self, rsin, x, dt):
        nc = self.nc
        rsout = self.dram("rsout", [TS, D], dt)
        nc.gpsimd.collective_compute(
            "ReduceScatter", OP.add, replica_groups=[list(range(NCORE))],
            ins=[rsin.ap()[:]], outs=[rsout.ap()[:]],
        )
        t = self.wk1.tile([P, D], dt, name=self.name("rsld"), tag="scrD")
        nc.sync.dma_start(t[:], rsout.ap()[:])
        nc.vector.tensor_add(out=x[:], in0=x[:], in1=t[:])

    # ---- main build --------------------------------------------------------
    def build(self):
        nc, tc = self.nc, self.tc
        with ExitStack() as st:
            self.ps = st.enter_context(tc.tile_pool(name="ps", bufs=8, space="PSUM"))
            self.big = st.enter_context(tc.tile_pool(name="big", bufs=2))
            self.sb = st.enter_context(tc.tile_pool(name="sb", bufs=1))
            self.wk = st.enter_context(tc.tile_pool(name="wk", bufs=2))
            self.wk1 = st.enter_context(tc.tile_pool(name="wk1", bufs=1))
            self.wr = st.enter_context(tc.tile_pool(name="wr", bufs=2))
            self.lmp = st.enter_context(tc.tile_pool(name="lmp", bufs=2))
            self.w1p = st.enter_context(tc.tile_pool(name="w1p", bufs=2))
            self.wt1 = st.enter_context(tc.tile_pool(name="wt1", bufs=1))
            self.sm = st.enter_context(tc.tile_pool(name="sm", bufs=3))
            self._build_inner()

    def _build_inner(self):
        nc, ia = self.nc, self.ia
        sb, wk, sm = self.sb, self.wk, self.sm

        self.idn = sb.tile([P, P], f32, name="idn")
        make_identity(nc, self.idn)
        self.eps = sb.tile([P, 1], f32, name="epsc")
        nc.vector.memset(self.eps[:], EPS)
        self.ones_fr = sb.tile([P, 1], fr, name="onesfrc")
        nc.sync.dma_start(self.ones_fr[:], ia["ones_fr"][:])
        self.ones16 = sb.tile([P, 1], f16, name="ones16c")
        nc.vector.memset(self.ones16[:], 1.0)
        self.onesrow_fr = sb.tile([1, P], fr, name="onesrowfrc")
        nc.sync.dma_start(self.onesrow_fr[:], ia["ones_row_fr"][:])
        self.onesrow16 = sb.tile([1, P], f16, name="onesrow16c")
        nc.vector.memset(self.onesrow16[:], 1.0)
        self.expb = sb.tile([P, 1], f32, name="expbc")
        nc.vector.memset(self.expb[:], -3.0)
        self.cc = sb.tile([P, T], f32, name="ccc")
        nc.sync.dma_start(self.cc[:], ia["cc"][:])
        self.ss = sb.tile([P, T], f32, name="ssc")
        nc.sync.dma_start(self.ss[:], ia["ss"][:])
        self.masks = sb.tile([P, NMASK, CH], f16, name="masksc")
        nc.sync.dma_start(self.masks[:], ia["masks"].rearrange("m p c -> p m c")[:])
        self.oh = sb.tile([NT, NT * P], f32, name="ohc")
        nc.sync.dma_start(self.oh[:], ia["oh"][:])
        self.lam_r = sb.tile([P, L], f32, name="lamrc")
        nc.sync.dma_start(self.lam_r[:], ia["lam_r"][:])
        self.lam_x = sb.tile([P, L], f32, name="lamxc")
        nc.sync.dma_start(self.lam_x[:], ia["lam_x"][:])
        self.wsel = sb.tile([P, E], f32, name="wselc")
        nc.sync.dma_start(self.wsel[:], ia["wsel"][:])
        self.idx_my = sb.tile([P, 1], i32, name="idxmyc")
        nc.sync.dma_start(self.idx_my[:], ia["idx_my"][:])
        self.idx_all = sb.tile([P, NT], i32, name="idxallc")
        nc.sync.dma_start(self.idx_all[:], ia["idx_all"][:])

        x0g = self.wk1.tile([P, D], f32, name="x0g", tag="scrD")
        nc.gpsimd.indirect_dma_start(
            out=x0g[:], out_offset=None, in_=ia["wte"][:],
            in_offset=IndirectOffsetOnAxis(ap=self.idx_my[:, :1], axis=0),
        )
        x0 = self.rms_tm(sb, "x0slot", x0g)
        x = sb.tile([P, D], f32, name="xres")
        nc.vector.tensor_copy(x[:], x0[:])

        for li in range(L):
            self.layer(li, x, x0)

        # final norm + lm_head (vocab-sharded, f16)
        xf = self.rms_tm(self.wk1, "rmsout", x)
        xf_fm, _, _ = self.allgather_fm(xf, "xf_fm", dt=f16)
        NVB = (VS + P - 1) // P
        for vb in range(NVB):
            vm = min(P, VS - vb * P)
            lw = self.lmp.tile([P, ND * P], f16, name=self.name("lm_wb"), tag="lmt")
            nc.sync.dma_start(lw[:], ia["lmh"][:, vb * ND * P:(vb + 1) * ND * P])
            pys = [self.ps.tile([P, CH], f32, name=self.name("ps_lm"), tag="ps")
                   for _ in range(NCH)]
            for db in range(ND):
                for ch in range(NCH):
                    nc.tensor.matmul(
                        pys[ch][:vm, :], lw[:, db * P:db * P + vm],
                        xf_fm[:, db, ch * CH:(ch + 1) * CH],
                        start=(db == 0), stop=(db == ND - 1),
                        skip_group_check=True,
                    )
            ot = wk.tile([P, T], f16, name=self.name("lm_o"), tag="lmstg")
            for ch in range(NCH):
                self.cp(ch, ot[:vm, ch * CH:(ch + 1) * CH], pys[ch][:vm, :])
            nc.sync.dma_start(ia["out"][vb * P:vb * P + vm, :], ot[:vm, :])

    # ---- one transformer layer ---------------------------------------------
    def layer(self, li, x, x0):
        nc, ia = self.nc, self.ia
        wk, sm = self.wk, self.sm
        plan = MASK_PLAN[WINDOWS[li]]
        moe_layer = li >= DENSE_N

        t1 = self.wk1.tile([P, D], f32, name=self.name("resmix"), tag="scrD")
        nc.vector.tensor_scalar(out=t1[:], in0=x0[:], scalar1=self.lam_x[:, li:li + 1],
                                scalar2=None, op0=OP.mult)
        nc.vector.scalar_tensor_tensor(out=x[:], in0=x[:], scalar=self.lam_r[:, li:li + 1],
                                       in1=t1[:], op0=OP.mult, op1=OP.add)

        # ---- attention (all f32r) -------------------------------------------
        xn = self.rms_tm(self.wk1, "rmsout", x)
        xn_fm, _, _ = self.allgather_fm(xn, f"xn_fm{li}", dt=fr)

        wqkv = self.wt1.tile([P, 3 * D], fr, name=self.name("wqkv"), tag="wqkv")
        nc.sync.dma_start(wqkv[:], ia["wqkv"][li])
        wo = self.wt1.tile([P, D], fr, name=self.name("wo_sb"), tag="wo")
        nc.sync.dma_start(wo[:], ia["wo"][li])

        def project(wi):
            """xn @ W[:, head] -> feature-major f32 [128hd, T].
            db outer so the stationary weight block is loaded once per db."""
            tag = "kfm" if wi == 1 else "qkv"
            o = self.wk1.tile([P, T], f32, name=self.name(f"prj{wi}"), tag=tag)
            pms = [self.ps.tile([P, CH], f32, name=self.name("ps_prj"), tag="ps")
                   for _ in range(NCH)]
            for db in range(ND):
                for ch in range(NCH):
                    nc.tensor.matmul(
                        pms[ch][:], wqkv[:, wi * D + db * P:wi * D + (db + 1) * P],
                        xn_fm[:, db, ch * CH:(ch + 1) * CH],
                        start=(db == 0), stop=(db == ND - 1),
                        skip_group_check=True,
                    )
            for ch in range(NCH):
                self.cp(ch, o[:, ch * CH:(ch + 1) * CH], pms[ch][:])
            return o

        def head_norm(t_fm, extra):
            sq = self.wk1.tile([P, T], f16, name=self.name("sq"), tag="sq")
            nc.vector.tensor_tensor(out=sq[:], in0=t_fm[:], in1=t_fm[:], op=OP.mult)
            pr = self.ps.tile([P, CH], f32, name=self.name("ps_hn"), tag="ps")
            for j in range(NT):
                nc.tensor.matmul(pr[:, j:j + 1], sq[:, j * P:(j + 1) * P],
                                 self.ones16[:], start=True, stop=True)
            s1 = sm.tile([P, NT], f32, name=self.name("hn1"), tag="smn")
            nc.scalar.activation(s1[:], pr[:, :NT], AF.Sqrt, bias=self.eps[:, :1],
                                 scale=1.0 / HD)
            s2 = sm.tile([P, NT], f32, name=self.name("hn2"), tag="smn")
            nc.vector.reciprocal(s2[:], s1[:])
            if extra != 1.0:
                nc.vector.tensor_scalar(out=s2[:], in0=s2[:], scalar1=extra,
                                        scalar2=None, op0=OP.mult)
            return s2

        def rope(t_fm, out):
            """rope on t_fm f32 [128, T]; final add (DVE) -> `out` tile."""
            tsw = self.wk1.tile([P, T], f32, name=self.name("tsw"), tag="tsw")
            nc.vector.tensor_copy(tsw[0:64, :], t_fm[64:128, :])
            nc.vector.tensor_copy(tsw[64:128, :], t_fm[0:64, :])
            nc.gpsimd.tensor_tensor(out=tsw[:], in0=tsw[:], in1=self.ss[:], op=OP.mult)
            nc.gpsimd.tensor_tensor(out=t_fm[:], in0=t_fm[:], in1=self.cc[:],
                                    op=OP.mult)
            eng = nc.gpsimd if out.dtype == f32 else nc.vector
            eng.tensor_add(out=out[:], in0=t_fm[:], in1=tsw[:])
            return out

        # q: project -> head-norm (HD^-0.5 folded) -> rope -> * rq -> f32r
        q_fm = project(0)
        rq_tm = head_norm(q_fm, float(HD) ** -0.5)
        qa = rope(q_fm, q_fm)
        ptr = self.ps.tile([P, CH], f32, name=self.name("ps_rqT"), tag="ps")
        nc.tensor.transpose(ptr[:NT, :P], rq_tm[:, :NT], self.idn[:])
        rqT = sm.tile([NT, P], f32, name=self.name("rqT"), tag="rqT")
        nc.scalar.copy(rqT[:], ptr[:NT, :P])
        qh = self.wk1.tile([P, T], fr, name=self.name("qh"), tag="qh")
        for hf in range(2):
            pq = self.ps.tile([P, CH], f32, name=self.name("ps_rqb"), tag="ps")
            for jj in range(NT // 2):
                j = hf * (NT // 2) + jj
                nc.tensor.matmul(pq[:, jj * P:(jj + 1) * P],
                                 self.oh[:, j * P:(j + 1) * P], rqT[:],
                                 start=True, stop=True)
            nc.vector.tensor_tensor(out=qh[:, hf * CH:(hf + 1) * CH],
                                    in0=qa[:, hf * CH:(hf + 1) * CH],
                                    in1=pq[:], op=OP.mult)
        # k
        k_fm = project(1)
        rk_tm = head_norm(k_fm, 1.0)
        kh = self.wk1.tile([P, T], fr, name=self.name("kh"), tag="kh")
        rope(k_fm, kh)

        # v (+ value embeddings)
        if li in VE_LAYERS:
            vj = VE_LAYERS[li]
            ve_tm = self.wk1.tile([P, NT, P], f32, name=self.name("ve_tm"), tag="tsw")
            for j in range(NT):
                nc.gpsimd.indirect_dma_start(
                    out=ve_tm[:, j, :], out_offset=None, in_=ia[f"ve{vj}"][:],
                    in_offset=IndirectOffsetOnAxis(ap=self.idx_all[:, j:j + 1], axis=0),
                )
            gate_row = self.wk1.tile([1, T], f32, name=self.name("gate_row"), tag="row")
            vegw = self.wt1.tile([P, 1], fr, name=self.name("vegwb"), tag="wblk1")
            nc.sync.dma_start(vegw[:], ia["vegw"][vj])
            for ch in range(NCH):
                pg = self.ps.tile([P, CH], f32, name=self.name("ps_vg"), tag="ps")
                nc.tensor.matmul(pg[0:1, :], vegw[:],
                                 xn_fm[:, 0, ch * CH:(ch + 1) * CH],
                                 start=True, stop=True)
                nc.scalar.activation(gate_row[:, ch * CH:(ch + 1) * CH], pg[0:1, :],
                                     AF.Sigmoid)
            nc.vector.tensor_scalar(out=gate_row[:], in0=gate_row[:], scalar1=2.0,
                                    scalar2=None, op0=OP.mult)
            gate_tm = self.row_to_tm(gate_row)
        else:
            ve_tm, gate_tm = None, None

        v_fm = project(2)
        v_tm = self.wk1.tile([P, NT, P], fr, name=self.name("v_tm"), tag="vtm")
        for j in range(NT):
            pt = self.ps.tile([P, CH], f32, name=self.name("ps_vt"), tag="ps")
            nc.tensor.transpose(pt[:, :P], v_fm[:, j * P:(j + 1) * P], self.idn[:])
            if ve_tm is None:
                self.cp(j, v_tm[:, j, :], pt[:, :P])
            else:
                nc.vector.scalar_tensor_tensor(
                    out=v_tm[:, j, :], in0=ve_tm[:, j, :], scalar=gate_tm[:, j:j + 1],
                    in1=pt[:, :P], op0=OP.mult, op1=OP.add)

        # scores -> exp -> (den, pv) interleaved; p_j streamed per (ch, j)
        den_row = self.wk1.tile([1, T], f32, name=self.name("den_row"), tag="row")
        y_sb = self.wk1.tile([P, T], fr, name=self.name("y_sb"), tag="qkv")
        for ch in range(NCH):
            live = [j for j in range(NT) if plan[(j, ch)] != "skip"]
            # PSUM ring is 8 slots; pd+py persist, so <=6 pm tiles per group
            groups = [live[i:i + 6] for i in range(0, len(live), 6)]
            ds = den_row[:, ch * CH:(ch + 1) * CH]
            ys = y_sb[:, ch * CH:(ch + 1) * CH]
            for gi, grp in enumerate(groups):
                pd = self.ps.tile([P, CH], f32, name=self.name("ps_den"), tag="ps")
                py = self.ps.tile([P, CH], f32, name=self.name("ps_pv"), tag="ps")
                for n, j in enumerate(grp):
                    kind = plan[(j, ch)]
                    pm = self.ps.tile([P, CH], f32, name=self.name("ps_sc"), tag="ps")
                    nc.tensor.matmul(pm[:], kh[:, j * P:(j + 1) * P],
                                     qh[:, ch * CH:(ch + 1) * CH],
                                     start=True, stop=True)
                    pj = wk.tile([P, CH], fr, name=self.name("p_j"), tag="pj")
                    nc.scalar.activation(pj[:], pm[:], AF.Exp,
                                         scale=rk_tm[:, j:j + 1],
                                         bias=self.expb[:, :1])
                    if kind != "full":
                        nc.vector.tensor_tensor(out=pj[:], in0=pj[:],
                                                in1=self.masks[:, kind, :],
                                                op=OP.mult)
                    nc.tensor.matmul(pd[0:1, :], self.ones_fr[:], pj[:],
                                     start=(n == 0), stop=(n == len(grp) - 1),
                                     skip_group_check=True)
                    nc.tensor.matmul(py[:], v_tm[:, j, :], pj[:],
                                     start=(n == 0), stop=(n == len(grp) - 1),
                                     skip_group_check=True)
                if gi == 0:
                    nc.scalar.copy(ds, pd[0:1, :])
                    self.cp(ch, ys, py[:])
                else:
                    nc.vector.tensor_add(out=ds, in0=ds, in1=pd[0:1, :])
                    nc.vector.tensor_add(out=ys, in0=ys, in1=py[:])
        den_tm = self.row_to_tm(den_row)
        rden_tm = sm.tile([P, NT], f32, name=self.name("rden"), tag="smn")
        nc.vector.reciprocal(rden_tm[:], den_tm[:])

        # o-proj partial (token-major; 1/den folded) -> rsin -> RS (f32)
        rsin = self.dram("rsin_a", [T, D], f32)
        for tb in range(NT):
            for ch in range(NCH):
                po = self.ps.tile([P, CH], f32, name=self.name("ps_op"), tag="ps")
                nc.tensor.matmul(po[:], y_sb[:, tb * P:(tb + 1) * P],
                                 wo[:, ch * CH:(ch + 1) * CH], start=True, stop=True)
                ot = wk.tile([P, CH], f32, name=self.name("o_stg"), tag="stg32")
                nc.scalar.mul(ot[:], po[:], rden_tm[:, tb:tb + 1])
                nc.sync.dma_start(
                    rsin.ap()[tb * P:(tb + 1) * P, ch * CH:(ch + 1) * CH], ot[:])
        self.reduce_scatter_add(rsin, x, f32)

        # ---- MLP / MoE ------------------------------------------------------
        xm = self.rms_tm(self.wk1, "rmsout", x)
        lowp = li == L - 1
        xm_fm, xmT, xmT32 = self.allgather_fm(
            xm, f"xm_fm{li}", dt=f16 if lowp else fr, want_f32T=lowp and moe_layer)
        rsin2 = self.dram("rsin_m", [T, D], f16 if lowp else f32)
        if not moe_layer:
            self.dense_mlp(li, xm_fm, rsin2)
        else:
            rt = xmT32 if lowp else xmT
            self.moe(li - DENSE_N, xm_fm, rt, rsin2, f16 if lowp else fr)
        self.reduce_scatter_add(rsin2, x, f16 if lowp else f32)

    # ---- dense mlp (F-sharded 512 per core; f32r) ---------------------------
    def dense_mlp(self, li, xm_fm, rsin2):
        nc, ia, wk = self.nc, self.ia, self.wk
        h2 = self.big.tile([P, NF, T], fr, name=self.name("h2"), tag="big")
        for fb in range(NF):
            w1s = self.w1p.tile([P, ND * P], fr, name=self.name("fcs"), tag="w1s")
            nc.sync.dma_start(w1s[:], ia["fc_s"][li, :, fb * D:(fb + 1) * D])
            pms = [self.ps.tile([P, CH], f32, name=self.name("ps_fc"), tag="ps")
                   for _ in range(NCH)]
            for db in range(ND):
                for ch in range(NCH):
                    nc.tensor.matmul(
                        pms[ch][:], w1s[:, db * P:(db + 1) * P],
                        xm_fm[:, db, ch * CH:(ch + 1) * CH],
                        start=(db == 0), stop=(db == ND - 1),
                        skip_group_check=True)
            for ch in range(NCH):
                rl = wk.tile([P, CH], fr, name=self.name("rl"), tag="sg")
                nc.scalar.activation(rl[:], pms[ch][:], AF.Relu)
                nc.vector.tensor_tensor(out=h2[:, fb, ch * CH:(ch + 1) * CH],
                                        in0=rl[:], in1=rl[:], op=OP.mult)
        for ch in range(NCH):
            psums = [self.ps.tile([P, CH], f32, name=self.name("ps_pj"), tag="ps")
                     for _ in range(NT)]
            for fb in range(NF):
                wb = self.wr.tile([P, CH], fr, name=self.name("pj_wb"), tag="wrs")
                nc.sync.dma_start(wb[:], ia["proj_s"][li, :,
                                                      fb * D + ch * CH:fb * D + (ch + 1) * CH])
                for tb in range(NT):
                    nc.tensor.matmul(psums[tb][:], h2[:, fb, tb * P:(tb + 1) * P],
                                     wb[:], start=(fb == 0), stop=(fb == NF - 1))
            for tb in range(NT):
                ot = wk.tile([P, CH], f32, name=self.name("pj_stg"), tag="stg32")
                self.cp(tb, ot[:], psums[tb][:])
                nc.sync.dma_start(
                    rsin2.ap()[tb * P:(tb + 1) * P, ch * CH:(ch + 1) * CH], ot[:])

    # ---- MoE (expert-sharded; dense over all tokens) ------------------------
    def moe(self, mi, xm_fm, xmT, rsin2, mdt):
        nc, ia, wk, sm = self.nc, self.ia, self.wk, self.sm
        # routing in fp32 on my resident tokens, then tiny AG (overlapped)
        rw_sb = sm.tile([P, ND, E], f32, name=self.name("rw_sb"), tag="rw")
        nc.sync.dma_start(rw_sb[:], ia["rw"][mi].rearrange("(n p) e -> p n e", p=P)[:])
        pr = self.ps.tile([P, CH], f32, name=self.name("ps_rt"), tag="ps")
        for db in range(ND):
            lh = xmT[:, db, :]
            if lh.dtype != f32:
                lh = lh.bitcast(f32)
            nc.tensor.matmul(pr[:, :E], lh, rw_sb[:, db, :],
                             start=(db == 0), stop=(db == ND - 1))
        nmax = sm.tile([P, 1], f32, name=self.name("nmax"), tag="sm1")
        nc.vector.tensor_reduce(nmax[:], pr[:, :E], axis=AX.X, op=OP.max, negate=True)
        probs = sm.tile([P, E], f32, name=self.name("probs"), tag="smn")
        se = sm.tile([P, 1], f32, name=self.name("se"), tag="sm1")
        nc.scalar.activation(probs[:], pr[:, :E], AF.Exp, bias=nmax[:, :1],
                             accum_out=se[:, :1])
        rse = sm.tile([P, 1], f32, name=self.name("rse"), tag="sm1")
        nc.vector.reciprocal(rse[:], se[:])
        nc.vector.tensor_scalar(out=probs[:], in0=probs[:], scalar1=rse[:, :1],
                                scalar2=None, op0=OP.mult)
        m8 = sm.tile([P, 8], f32, name=self.name("m8"), tag="smn")
        nc.vector.max(m8[:], probs[:])
        wf_my = sm.tile([P, E], f32, name=self.name("wfmy"), tag="smn")
        nc.vector.tensor_scalar(out=wf_my[:], in0=probs[:], scalar1=m8[:, 1:2],
                                scalar2=None, op0=OP.is_ge)
        nc.vector.tensor_tensor(out=wf_my[:], in0=wf_my[:], in1=probs[:], op=OP.mult)
        wfin = self.dram("wfin", [TS, E])
        nc.sync.dma_start(wfin.ap()[:], wf_my[:])
        wfout = self.dram("wfout", [T, E], shared=True)
        nc.gpsimd.collective_compute(
            "AllGather", OP.bypass, replica_groups=[list(range(NCORE))],
            ins=[wfin.ap()[:]], outs=[wfout.ap()[:]],
        )
        wf_all = sm.tile([P, NT, E], f32, name=self.name("wfall"), tag="wfall")
        nc.sync.dma_start(wf_all[:], wfout.ap().rearrange("(j p) e -> p j e", p=P)[:])
        wcol = sm.tile([P, NT], f32, name=self.name("wcol"), tag="wcol")
        wfsel = sm.tile([P, NT, E], f32, name=self.name("wfsel"), tag="wfall")
        nc.vector.tensor_tensor(out=wfsel[:], in0=wf_all[:],
                                in1=self.wsel[:, None, :].to_broadcast([P, NT, E]),
                                op=OP.mult)
        nc.vector.tensor_reduce(wcol[:], wfsel[:], axis=AX.X, op=OP.add)

        gut = self.wt1.tile([P, ND * 2 * P], mdt, name=self.name("gut"), tag="gut")
        nc.sync.dma_start(gut[:], ia[f"gu_s_{mi}"][:])
        down = self.wt1.tile([P, D], mdt, name=self.name("down_sb"), tag="wo")
        nc.sync.dma_start(down[:], ia[f"down_s_{mi}"][:])
        gwt = self.wt1.tile([P, ND], mdt, name=self.name("gwt"), tag="gwt")
        nc.sync.dma_start(gwt[:], ia[f"gatew_{mi}"][:])
        onesrow = self.onesrow16 if mdt == f16 else self.onesrow_fr

        # shared expert: su = sig(gate) * silu(g) * u
        gt_row = self.wk1.tile([1, T], mdt, name=self.name("gt_row"), tag="row")
        for ch in range(NCH):
            pg = self.ps.tile([P, CH], f32, name=self.name("ps_sg"), tag="ps")
            for db in range(ND):
                nc.tensor.matmul(pg[0:1, :], gwt[:, db:db + 1],
                                 xm_fm[:, db, ch * CH:(ch + 1) * CH],
                                 start=(db == 0), stop=(db == ND - 1))
            nc.scalar.activation(gt_row[:, ch * CH:(ch + 1) * CH], pg[0:1, :],
                                 AF.Sigmoid)
        su = self.wk1.tile([P, T], mdt, name=self.name("su_sb"), tag="su")
        for ch in range(NCH):
            bc = self.ps.tile([P, CH], f32, name=self.name("ps_bc"), tag="ps")
            nc.tensor.matmul(bc[:], onesrow[:],
                             gt_row[:, ch * CH:(ch + 1) * CH], start=True, stop=True)
            pm_g = self.ps.tile([P, CH], f32, name=self.name("ps_g"), tag="ps")
            for db in range(ND):
                nc.tensor.matmul(pm_g[:], gut[:, db * 2 * P:db * 2 * P + P],
                                 xm_fm[:, db, ch * CH:(ch + 1) * CH],
                                 start=(db == 0), stop=(db == ND - 1))
            pm_u = self.ps.tile([P, CH], f32, name=self.name("ps_u"), tag="ps")
            for db in range(ND):
                nc.tensor.matmul(pm_u[:], gut[:, db * 2 * P + P:(db + 1) * 2 * P],
                                 xm_fm[:, db, ch * CH:(ch + 1) * CH],
                                 start=(db == 0), stop=(db == ND - 1))
            cs = slice(ch * CH, (ch + 1) * CH)
            sg = wk.tile([P, CH], mdt, name=self.name("sg_stg"), tag="sg")
            nc.scalar.activation(sg[:], pm_g[:], AF.Silu)
            nc.vector.tensor_tensor(out=sg[:], in0=sg[:], in1=pm_u[:], op=OP.mult)
            nc.vector.tensor_tensor(out=su[:, cs], in0=sg[:], in1=bc[:], op=OP.mult)

        # routed expert: h = silu(xm @ w1)
        h = self.big.tile([P, ND, T], mdt, name=self.name("h_moe"), tag="big")
        for fb in range(ND):
            w1s = self.w1p.tile([P, ND * P], mdt, name=self.name("w1s"), tag="w1s")
            nc.sync.dma_start(w1s[:], ia[f"w1_{mi}"][:, fb * D:(fb + 1) * D])
            pms = [self.ps.tile([P, CH], f32, name=self.name("ps_w1"), tag="ps")
                   for _ in range(NCH)]
            for db in range(ND):
                for ch in range(NCH):
                    nc.tensor.matmul(pms[ch][:], w1s[:, db * P:(db + 1) * P],
                                     xm_fm[:, db, ch * CH:(ch + 1) * CH],
                                     start=(db == 0), stop=(db == ND - 1),
                                     skip_group_check=True)
            for ch in range(NCH):
                cs = slice(ch * CH, (ch + 1) * CH)
                nc.scalar.activation(h[:, fb, cs], pms[ch][:], AF.Silu)

        # y = wcol * (h @ w2) + su @ down -> rsin2
        odt = f16 if mdt == f16 else f32
        for ch in range(NCH):
            psums = [self.ps.tile([P, CH], f32, name=self.name("ps_w2"), tag="ps")
                     for _ in range(NT)]
            for fb in range(ND):
                wb = self.wr.tile([P, CH], mdt, name=self.name("w2wb"), tag="wrs")
                nc.sync.dma_start(wb[:], ia[f"w2_{mi}"][:,
                                                        fb * D + ch * CH:fb * D + (ch + 1) * CH])
                for tb in range(NT):
                    nc.tensor.matmul(psums[tb][:], h[:, fb, tb * P:(tb + 1) * P],
                                     wb[:], start=(fb == 0), stop=(fb == ND - 1))
            for tb in range(NT):
                ot = wk.tile([P, CH], odt, name=self.name("moe_stg"),
                             tag="stg" if odt == f16 else "stg32")
                nc.scalar.mul(ot[:], psums[tb][:], wcol[:, tb:tb + 1])
                pd = self.ps.tile([P, CH], f32, name=self.name("ps_dn"), tag="ps")
                nc.tensor.matmul(pd[:], su[:, tb * P:(tb + 1) * P],
                                 down[:, ch * CH:(ch + 1) * CH], start=True, stop=True)
                nc.vector.tensor_add(out=ot[:], in0=ot[:], in1=pd[:])
                nc.sync.dma_start(
                    rsin2.ap()[tb * P:(tb + 1) * P, ch * CH:(ch + 1) * CH], ot[:])


# ---------------------------------------------------------------- build + run
_BUILT = None


def _build():
    global _BUILT
    if _BUILT is not None:
        return _BUILT
    nc = bacc.Bacc("TRN2", target_bir_lowering=False, debug=False, num_devices=NCORE)

    def inp(name, shape, dtype=f32):
        return nc.dram_tensor(name, list(shape), dtype, kind="ExternalInput").ap()

    ia = {
        "idx_my": inp("idx_my", [P, 1], i32),
        "idx_all": inp("idx_all", [P, NT], i32),
        "wte": inp("wte", [V, D]),
        "ve0": inp("ve0", [V, P]),
        "ve1": inp("ve1", [V, P]),
        "vegw": inp("vegw", [2, P, 1], fr),
        "ones_fr": inp("ones_fr", [P, 1], fr),
        "ones_row_fr": inp("ones_row_fr", [1, P], fr),
        "wqkv": inp("wqkv", [L, P, 3 * D], fr),
        "wo": inp("wo", [L, P, D], fr),
        "fc_s": inp("fc_s", [DENSE_N, P, NF * D], fr),
        "proj_s": inp("proj_s", [DENSE_N, P, NF * D], fr),
        "rw": inp("rw", [2, D, E]),
        "wsel": inp("wsel", [P, E]),
        "w1_0": inp("w1_0", [P, ND * D], fr),
        "w1_1": inp("w1_1", [P, ND * D], f16),
        "w2_0": inp("w2_0", [P, ND * D], fr),
        "w2_1": inp("w2_1", [P, ND * D], f16),
        "gu_s_0": inp("gu_s_0", [P, ND * 2 * P], fr),
        "gu_s_1": inp("gu_s_1", [P, ND * 2 * P], f16),
        "down_s_0": inp("down_s_0", [P, D], fr),
        "down_s_1": inp("down_s_1", [P, D], f16),
        "gatew_0": inp("gatew_0", [P, ND], fr),
        "gatew_1": inp("gatew_1", [P, ND], f16),
        "lmh": inp("lmh", [P, ((VS + P - 1) // P) * ND * P], f16),
        "lam_r": inp("lam_r", [P, L]),
        "lam_x": inp("lam_x", [P, L]),
        "cc": inp("cc", [P, T]),
        "ss": inp("ss", [P, T]),
        "masks": inp("masks", [NMASK, P, CH], f16),
        "oh": inp("oh", [NT, NT * P]),
        "out": nc.dram_tensor("out", [VS, T], f16, kind="ExternalOutput").ap(),
    }
    with tile.TileContext(nc) as tc:
        Builder(nc, tc, ia).build()
    nc.compile()
    _BUILT = nc
    return nc


def _h(a):
    return np.ascontiguousarray(np.asarray(a)).astype(NPH)


def _f(a):
    return np.ascontiguousarray(np.asarray(a), dtype=np.float32)


def make_in_maps(inputs):
    idx = np.asarray(inputs["idx"]).reshape(T).astype(np.int32)
    cc, ss = _rope_tables()
    oh = np.zeros((NT, NT * P), np.float32)
    for j in range(NT):
        oh[j, j * P:(j + 1) * P] = 1.0
    shared = {
        "idx_all": np.ascontiguousarray(idx.reshape(NT, P).T),
        "wte": _f(inputs["wte"]),
        "rw": _f(inputs["router_w"]),
        "lam_r": np.ascontiguousarray(
            np.broadcast_to(np.asarray(inputs["resid_lambdas"], np.float32), (P, L))),
        "lam_x": np.ascontiguousarray(
            np.broadcast_to(np.asarray(inputs["x0_lambdas"], np.float32), (P, L))),
        "cc": cc,
        "ss": ss,
        "ones_fr": np.ones((P, 1), np.float32),
        "ones_row_fr": np.ones((1, P), np.float32),
        "masks": MASKS.astype(NPH),
        "oh": oh,
    }
    wq = np.asarray(inputs["attn_q"])
    wk_ = np.asarray(inputs["attn_k"])
    wv = np.asarray(inputs["attn_v"])
    gu_full = np.asarray(inputs["shared_gu"])
    in_maps = []
    for c in range(NCORE):
        hs = slice(c * P, (c + 1) * P)
        vegw = np.zeros((2, P, 1), np.float32)
        for j in range(2):
            vegw[j, :VE_GATE_CH, 0] = np.asarray(inputs["ve_gate_w"])[j][:, c]
        wqkv = np.empty((L, P, 3 * D), np.float32)
        for l in range(L):
            wqkv[l, :, 0 * D:1 * D] = _pack(_f(wq[l][:, hs]), P)
            wqkv[l, :, 1 * D:2 * D] = _pack(_f(wk_[l][:, hs]), P)
            wqkv[l, :, 2 * D:3 * D] = _pack(_f(wv[l][:, hs]), P)
        fc_s = np.stack([
            _pack_fb(_f(np.asarray(inputs["mlp_fc"])[l][:, c * 512:(c + 1) * 512]), NF)
            for l in range(DENSE_N)])
        proj_s = np.stack([
            _pack(_f(np.asarray(inputs["mlp_proj"])[l][c * 512:(c + 1) * 512, :]), P)
            for l in range(DENSE_N)])
        gu_c = np.empty((2, D, 2 * P), np.float32)
        for m in range(2):
            gu_c[m, :, :P] = gu_full[m][:, c * P:(c + 1) * P]
            gu_c[m, :, P:] = gu_full[m][:, F + c * P:F + (c + 1) * P]
        wsel = np.zeros((P, E), np.float32)
        wsel[:, c] = 1.0
        m = dict(shared)
        m.update({
            "idx_my": np.ascontiguousarray(idx[c * P:(c + 1) * P, None]),
            "ve0": _f(np.asarray(inputs["ve_tables"])[0][:, hs]),
            "ve1": _f(np.asarray(inputs["ve_tables"])[1][:, hs]),
            "vegw": vegw,
            "wqkv": wqkv,
            "wo": _f(np.asarray(inputs["attn_o"])[:, hs, :]),
            "fc_s": fc_s,
            "proj_s": proj_s,
            "wsel": wsel,
            "w1_0": _pack_fb(_f(np.asarray(inputs["moe_w1"])[0, c]), ND),
            "w1_1": _pack_fb(_h(np.asarray(inputs["moe_w1"])[1, c]), ND),
            "w2_0": _pack(_f(np.asarray(inputs["moe_w2"])[0, c]), P),
            "w2_1": _pack(_h(np.asarray(inputs["moe_w2"])[1, c]), P),
            "gu_s_0": _pack(_f(gu_c[0]), P),
            "gu_s_1": _pack(_h(gu_c[1]), P),
            "down_s_0": _f(np.asarray(inputs["shared_down"])[0, c * P:(c + 1) * P, :]),
            "down_s_1": _h(np.asarray(inputs["shared_down"])[1, c * P:(c + 1) * P, :]),
            "gatew_0": _f(np.asarray(inputs["shared_gate_w"])[0].reshape(ND, P).T),
            "gatew_1": _h(np.asarray(inputs["shared_gate_w"])[1].reshape(ND, P).T),
            "lmh": _pack_lmh(_h(np.asarray(inputs["lm_head_w"])[:, c * VS:(c + 1) * VS])),
        })
        in_maps.append(m)
    return in_maps


def kernel(**inputs):
    nc = _build()
    in_maps = make_in_maps(inputs)
    res = run_bass_kernel_spmd(nc, in_maps, list(range(NCORE)))
    outs = [res.results[c]["out"].astype(np.float32).T for c in range(NCORE)]
    return np.concatenate(outs, axis=1).reshape(B, T, V)


if __name__ == "__main__":
    nc = _build()
    n_inst = sum(len(bb.instructions) for bb in nc.main_func.blocks)
    print("build OK; instructions:", n_inst)



# revision 6
# speedup vs baseline: 4.1652x; 4.1652x over previous
"""Trainium2 Bass kernel for nn_AttentionMoeModel — v3 (f32r pre-routing).

Sharding as baseline. Speed comes from:
  - float32r matmuls (1 cy/row at >=256 moving dim) for everything upstream of
    the MoE routers — same precision as the HW fp32 path (~1.4e-4), which the
    razor-thin top-2 routing margins require — fp16 only post-routing
    (layer-3 FFN products + lm_head).
  - weights host-prepacked for few big DMAs; fc/proj/w1/w2 streamed per-block
  - scores/exp/den/pv interleaved per (ch, j): p never materialized fully
  - softmax 1/den folded into o-proj PSUM copy; rq broadcast via one-hot
    matmuls; wf AllGather overlapped with shared-expert/w1 compute
"""
import sys

sys.path.insert(0, "/opt/trn_rl_repo")

from contextlib import ExitStack

import numpy as np

import concourse.bass as bass
import concourse.mybir as mybir
import concourse.tile as tile
from concourse import bacc
from concourse.bass import IndirectOffsetOnAxis
from concourse.bass_utils import run_bass_kernel_spmd
from concourse.masks import make_identity

B, T, D, H, HD, V, L = 1, 1024, 1024, 8, 128, 32000, 4
E, F = 8, 1024
DENSE_N = 2
VE_LAYERS = {0: 0, 3: 1}
WINDOWS = [1024, 512, 1024, 1024]
VE_GATE_CH = 32

NCORE = 8
P = 128
TS = T // NCORE
ND = D // P
NT = T // P
VS = V // NCORE
CH = 512
NCH = T // CH
NF = 4
EPS = 1e-6

f32 = mybir.dt.float32
f16 = mybir.dt.float16
fr = mybir.dt.float32r
i32 = mybir.dt.int32
AF = mybir.ActivationFunctionType
OP = mybir.AluOpType
AX = mybir.AxisListType
NPH = np.float16


def _rope_tables():
    inv = 1.0 / (10000.0 ** (np.arange(0, HD, 2, dtype=np.float64) / HD))
    fri = np.arange(T, dtype=np.float64)[:, None] * inv[None, :]
    cos, sin = np.cos(fri), np.sin(fri)
    cc = np.empty((P, T), np.float32)
    ss = np.empty((P, T), np.float32)
    cc[:64] = cos.T
    cc[64:] = cos.T
    ss[:64] = sin.T
    ss[64:] = -sin.T
    return cc, ss


def _block_mask(w, j, ch):
    tk = np.arange(P)[:, None] + P * j
    tq = np.arange(CH)[None, :] + CH * ch
    return ((tk <= tq) & (tq - tk <= w)).astype(np.float32)


def _mask_plan():
    uniq, keys, plan = [], {}, {}
    for w in set(WINDOWS):
        plan[w] = {}
        for j in range(NT):
            for ch in range(NCH):
                m = _block_mask(w, j, ch)
                if not m.any():
                    plan[w][(j, ch)] = "skip"
                elif m.all():
                    plan[w][(j, ch)] = "full"
                else:
                    kb = m.tobytes()
                    if kb not in keys:
                        keys[kb] = len(uniq)
                        uniq.append(m)
                    plan[w][(j, ch)] = keys[kb]
    return np.stack(uniq), plan


MASKS, MASK_PLAN = _mask_plan()
NMASK = MASKS.shape[0]


def _pack(w, blk):
    """[Kb*blk, C] -> [blk, Kb*C] with pack[p, kb*C+c] = w[kb*blk+p, c]."""
    kb = w.shape[0] // blk
    return np.ascontiguousarray(
        w.reshape(kb, blk, w.shape[1]).transpose(1, 0, 2).reshape(blk, kb * w.shape[1]))


def _pack_lmh(w):
    """[D, VS] -> [128, NVB*ND*128]: out[p, (vb*ND+db)*128+c] =
    w[db*128+p, vb*128+c], zero-padded in vb tail."""
    nvb = (VS + P - 1) // P
    out = np.zeros((P, nvb * ND * P), w.dtype)
    for vb in range(nvb):
        vm = min(P, VS - vb * P)
        blk = w[:, vb * P:vb * P + vm]            # [D, vm]
        r = blk.reshape(ND, P, vm)                # [db, p, c]
        for db in range(ND):
            out[:, (vb * ND + db) * P:(vb * ND + db) * P + vm] = r[db]
    return out


def _pack_fb(w, nfb):
    """[D, Fb*128] -> [128, nfb * (ND*128)]: out[p, fb*D + db*128 + c] =
    w[db*128+p, fb*128+c]  (per-fb stationary tiles for w1/fc)."""
    Din = w.shape[0]
    nd = Din // P
    r = w.reshape(nd, P, nfb, P).transpose(1, 2, 0, 3).reshape(P, nfb * nd * P)
    return np.ascontiguousarray(r)


class Builder:
    def __init__(self, nc, tc, ia):
        self.nc = nc
        self.tc = tc
        self.ia = ia
        self.uid = 0

    def name(self, s):
        self.uid += 1
        return f"{s}_{self.uid}"

    def dram(self, s, shape, dtype=f32, shared=False):
        if shared:
            return self.nc.dram_tensor(self.name(s), shape, dtype, addr_space="Shared")
        return self.nc.dram_tensor(self.name(s), shape, dtype)

    # ---- small helpers -----------------------------------------------------
    def cp(self, i, out, in_):
        if i % 2 == 0:
            self.nc.scalar.copy(out, in_)
        else:
            self.nc.vector.tensor_copy(out, in_)

    def rms_tm(self, out_pool, out_tag, x):
        nc = self.nc
        scr = self.wk1.tile([P, D], f16, name=self.name("rms_scr"), tag="sq")
        ssq = self.sm.tile([P, 1], f32, name=self.name("ssq"), tag="sm1")
        nc.scalar.activation(scr[:], x[:], AF.Square, accum_out=ssq[:, :1])
        s1 = self.sm.tile([P, 1], f32, name=self.name("rms_s1"), tag="sm1")
        nc.scalar.activation(s1[:], ssq[:], AF.Sqrt, bias=self.eps[:, :1], scale=1.0 / D)
        s2 = self.sm.tile([P, 1], f32, name=self.name("rms_s2"), tag="sm1")
        nc.vector.reciprocal(s2[:], s1[:])
        xn = out_pool.tile([P, D], f32, name=self.name("rms_out"), tag=out_tag)
        nc.scalar.mul(xn[:], x[:], s2[:, :1])
        return xn

    def row_to_tm(self, row):
        nc = self.nc
        db = self.dram("tb", [1, NT * P])
        nc.sync.dma_start(db.ap()[:], row[:])
        out = self.sm.tile([P, NT], f32, name=self.name("tmn"), tag="smn")
        nc.sync.dma_start(out[:], db.ap().rearrange("o (j p) -> (o p) j", p=P)[:])
        return out

    def allgather_fm(self, xn, nm, dt=fr, want_f32T=False):
        """xn [128, D] tm fp32 -> x_fm [128, ND, T] (dt) in 'big' pool."""
        nc = self.nc
        xnT = self.wk1.tile([P, ND, TS], dt, name=self.name("xnT"), tag="xnT")
        xnT32 = None
        if want_f32T:
            xnT32 = self.wk1.tile([P, ND, TS], f32, name=self.name("xnT32"),
                                  tag="xnT32")
        for db in range(ND):
            pt = self.ps.tile([P, CH], f32, name=self.name("ps_tr"), tag="ps")
            nc.tensor.transpose(pt[:, :P], xn[:, db * P:(db + 1) * P], self.idn[:])
            self.cp(db, xnT[:, db, :], pt[:, :P])
            if want_f32T:
                nc.vector.tensor_copy(xnT32[:, db, :], pt[:, :P])
        agin = self.dram("agin", [D, TS], dt)
        nc.sync.dma_start(agin.ap().rearrange("(n p) t -> p n t", p=P)[:], xnT[:])
        agout = self.dram("agout", [NCORE * D, TS], dt, shared=True)
        nc.gpsimd.collective_compute(
            "AllGather", OP.bypass, replica_groups=[list(range(NCORE))],
            ins=[agin.ap()[:]], outs=[agout.ap()[:]],
        )
        x_fm = self.big.tile([P, ND, T], dt, name=self.name(nm), tag="big")
        agv = agout.ap().rearrange("(r n p) t -> p n r t", p=P, n=ND)
        for db in range(ND):
            nc.sync.dma_start(
                x_fm.rearrange("p n (r t) -> p n r t", r=NCORE)[:, db, :, :],
                agv[:, db, :, :],
            )
        return x_fm, xnT, xnT32

    def reduce_scatter_add(self, rsin, x, dt):
        nc = self.nc
        rsout = self.dram("rsout", [TS, D], dt)
        nc.gpsimd.collective_compute(
            "ReduceScatter", OP.add, replica_groups=[list(range(NCORE))],
            ins=[rsin.ap()[:]], outs=[rsout.ap()[:]],
        )
        t = self.wk1.tile([P, D], dt, name=self.name("rsld"), tag="scrD")
        nc.sync.dma_start(t[:], rsout.ap()[:])
        nc.vector.tensor_add(out=x[:], in0=x[:], in1=t[:])

    # ---- main build --------------------------------------------------------
    def build(self):
        nc, tc = self.nc, self.tc
        with ExitStack() as st:
            self.ps = st.enter_context(tc.tile_pool(name="ps", bufs=8, space="PSUM"))
            self.big = st.enter_context(tc.tile_pool(name="big", bufs=2))
            self.sb = st.enter_context(tc.tile_pool(name="sb", bufs=1))
            self.wk = st.enter_context(tc.tile_pool(name="wk", bufs=2))
            self.wk1 = st.enter_context(tc.tile_pool(name="wk1", bufs=1))
            self.wr = st.enter_context(tc.tile_pool(name="wr", bufs=2))
            self.lmp = st.enter_context(tc.tile_pool(name="lmp", bufs=2))
            self.w1p = st.enter_context(tc.tile_pool(name="w1p", bufs=2))
            self.wt1 = st.enter_context(tc.tile_pool(name="wt1", bufs=1))
            self.sm = st.enter_context(tc.tile_pool(name="sm", bufs=3))
            self._build_inner()

    def _build_inner(self):
        nc, ia = self.nc, self.ia
        sb, wk, sm = self.sb, self.wk, self.sm

        self.idn = sb.tile([P, P], f32, name="idn")
        make_identity(nc, self.idn)
        self.eps = sb.tile([P, 1], f32, name="epsc")
        nc.vector.memset(self.eps[:], EPS)
        self.ones_fr = sb.tile([P, 1], fr, name="onesfrc")
        nc.sync.dma_start(self.ones_fr[:], ia["ones_fr"][:])
        self.ones16 = sb.tile([P, 1], f16, name="ones16c")
        nc.vector.memset(self.ones16[:], 1.0)
        self.onesrow_fr = sb.tile([1, P], fr, name="onesrowfrc")
        nc.sync.dma_start(self.onesrow_fr[:], ia["ones_row_fr"][:])
        self.onesrow16 = sb.tile([1, P], f16, name="onesrow16c")
        nc.vector.memset(self.onesrow16[:], 1.0)
        self.expb = sb.tile([P, 1], f32, name="expbc")
        nc.vector.memset(self.expb[:], -3.0)
        self.cc = sb.tile([P, T], f32, name="ccc")
        nc.sync.dma_start(self.cc[:], ia["cc"][:])
        self.ss = sb.tile([P, T], f32, name="ssc")
        nc.sync.dma_start(self.ss[:], ia["ss"][:])
        self.masks = sb.tile([P, NMASK, CH], f16, name="masksc")
        nc.sync.dma_start(self.masks[:], ia["masks"].rearrange("m p c -> p m c")[:])
        self.oh = sb.tile([NT, NT * P], f32, name="ohc")
        nc.sync.dma_start(self.oh[:], ia["oh"][:])
        self.lam_r = sb.tile([P, L], f32, name="lamrc")
        nc.sync.dma_start(self.lam_r[:], ia["lam_r"][:])
        self.lam_x = sb.tile([P, L], f32, name="lamxc")
        nc.sync.dma_start(self.lam_x[:], ia["lam_x"][:])
        self.wsel = sb.tile([P, E], f32, name="wselc")
        nc.sync.dma_start(self.wsel[:], ia["wsel"][:])
        self.idx_my = sb.tile([P, 1], i32, name="idxmyc")
        nc.sync.dma_start(self.idx_my[:], ia["idx_my"][:])
        self.idx_all = sb.tile([P, NT], i32, name="idxallc")
        nc.sync.dma_start(self.idx_all[:], ia["idx_all"][:])

        x0g = self.wk1.tile([P, D], f32, name="x0g", tag="scrD")
        nc.gpsimd.indirect_dma_start(
            out=x0g[:], out_offset=None, in_=ia["wte"][:],
            in_offset=IndirectOffsetOnAxis(ap=self.idx_my[:, :1], axis=0),
        )
        x0 = self.rms_tm(sb, "x0slot", x0g)
        x = sb.tile([P, D], f32, name="xres")
        nc.vector.tensor_copy(x[:], x0[:])

        for li in range(L):
            self.layer(li, x, x0)

        # final norm + lm_head (vocab-sharded, f16)
        xf = self.rms_tm(self.wk1, "rmsout", x)
        xf_fm, _, _ = self.allgather_fm(xf, "xf_fm", dt=f16)
        NVB = (VS + P - 1) // P
        for vb in range(NVB):
            vm = min(P, VS - vb * P)
            lw = self.lmp.tile([P, ND * P], f16, name=self.name("lm_wb"), tag="lmt")
            nc.sync.dma_start(lw[:], ia["lmh"][:, vb * ND * P:(vb + 1) * ND * P])
            pys = [self.ps.tile([P, CH], f32, name=self.name("ps_lm"), tag="ps")
                   for _ in range(NCH)]
            for db in range(ND):
                for ch in range(NCH):
                    nc.tensor.matmul(
                        pys[ch][:vm, :], lw[:, db * P:db * P + vm],
                        xf_fm[:, db, ch * CH:(ch + 1) * CH],
                        start=(db == 0), stop=(db == ND - 1),
                        skip_group_check=True,
                    )
            ot = wk.tile([P, T], f16, name=self.name("lm_o"), tag="lmstg")
            for ch in range(NCH):
                self.cp(ch, ot[:vm, ch * CH:(ch + 1) * CH], pys[ch][:vm, :])
            nc.sync.dma_start(ia["out"][vb * P:vb * P + vm, :], ot[:vm, :])

    # ---- one transformer layer ---------------------------------------------
    def layer(self, li, x, x0):
        nc, ia = self.nc, self.ia
        wk, sm = self.wk, self.sm
        plan = MASK_PLAN[WINDOWS[li]]
        moe_layer = li >= DENSE_N

        t1 = self.wk1.tile([P, D], f32, name=self.name("resmix"), tag="scrD")
        nc.vector.tensor_scalar(out=t1[:], in0=x0[:], scalar1=self.lam_x[:, li:li + 1],
                                scalar2=None, op0=OP.mult)
        nc.vector.scalar_tensor_tensor(out=x[:], in0=x[:], scalar=self.lam_r[:, li:li + 1],
                                       in1=t1[:], op0=OP.mult, op1=OP.add)

        # ---- attention (all f32r) -------------------------------------------
        xn = self.rms_tm(self.wk1, "rmsout", x)
        xn_fm, _, _ = self.allgather_fm(xn, f"xn_fm{li}", dt=fr)

        wqkv = self.wt1.tile([P, 3 * D], fr, name=self.name("wqkv"), tag="wqkv")
        nc.sync.dma_start(wqkv[:], ia["wqkv"][li])
        wo = self.wt1.tile([P, D], fr, name=self.name("wo_sb"), tag="wo")
        nc.sync.dma_start(wo[:], ia["wo"][li])

        def project(wi):
            """xn @ W[:, head] -> feature-major f32 [128hd, T].
            db outer so the stationary weight block is loaded once per db."""
            tag = "kfm" if wi == 1 else "qkv"
            o = self.wk1.tile([P, T], f32, name=self.name(f"prj{wi}"), tag=tag)
            pms = [self.ps.tile([P, CH], f32, name=self.name("ps_prj"), tag="ps")
                   for _ in range(NCH)]
            for db in range(ND):
                for ch in range(NCH):
                    nc.tensor.matmul(
                        pms[ch][:], wqkv[:, wi * D + db * P:wi * D + (db + 1) * P],
                        xn_fm[:, db, ch * CH:(ch + 1) * CH],
                        start=(db == 0), stop=(db == ND - 1),
                        skip_group_check=True,
                    )
            for ch in range(NCH):
                self.cp(ch, o[:, ch * CH:(ch + 1) * CH], pms[ch][:])
            return o

        def head_norm(t_fm, extra):
            sq = self.wk1.tile([P, T], f16, name=self.name("sq"), tag="sq")
            nc.vector.tensor_tensor(out=sq[:], in0=t_fm[:], in1=t_fm[:], op=OP.mult)
            pr = self.ps.tile([P, CH], f32, name=self.name("ps_hn"), tag="ps")
            for j in range(NT):
                nc.tensor.matmul(pr[:, j:j + 1], sq[:, j * P:(j + 1) * P],
                                 self.ones16[:], start=True, stop=True)
            s1 = sm.tile([P, NT], f32, name=self.name("hn1"), tag="smn")
            nc.scalar.activation(s1[:], pr[:, :NT], AF.Sqrt, bias=self.eps[:, :1],
                                 scale=1.0 / HD)
            s2 = sm.tile([P, NT], f32, name=self.name("hn2"), tag="smn")
            nc.vector.reciprocal(s2[:], s1[:])
            if extra != 1.0:
                nc.vector.tensor_scalar(out=s2[:], in0=s2[:], scalar1=extra,
                                        scalar2=None, op0=OP.mult)
            return s2

        def rope(t_fm, out):
            """rope on t_fm f32 [128, T]; final add (DVE) -> `out` tile."""
            tsw = self.wk1.tile([P, T], f32, name=self.name("tsw"), tag="tsw")
            nc.vector.tensor_copy(tsw[0:64, :], t_fm[64:128, :])
            nc.vector.tensor_copy(tsw[64:128, :], t_fm[0:64, :])
            nc.gpsimd.tensor_tensor(out=tsw[:], in0=tsw[:], in1=self.ss[:], op=OP.mult)
            nc.gpsimd.tensor_tensor(out=t_fm[:], in0=t_fm[:], in1=self.cc[:],
                                    op=OP.mult)
            eng = nc.gpsimd if out.dtype == f32 else nc.vector
            eng.tensor_add(out=out[:], in0=t_fm[:], in1=tsw[:])
            return out

        # q: project -> head-norm (HD^-0.5 folded) -> rope -> * rq -> f32r
        q_fm = project(0)
        rq_tm = head_norm(q_fm, float(HD) ** -0.5)
        qa = rope(q_fm, q_fm)
        ptr = self.ps.tile([P, CH], f32, name=self.name("ps_rqT"), tag="ps")
        nc.tensor.transpose(ptr[:NT, :P], rq_tm[:, :NT], self.idn[:])
        rqT = sm.tile([NT, P], f32, name=self.name("rqT"), tag="rqT")
        nc.scalar.copy(rqT[:], ptr[:NT, :P])
        qh = self.wk1.tile([P, T], fr, name=self.name("qh"), tag="qh")
        for hf in range(2):
            pq = self.ps.tile([P, CH], f32, name=self.name("ps_rqb"), tag="ps")
            for jj in range(NT // 2):
                j = hf * (NT // 2) + jj
                nc.tensor.matmul(pq[:, jj * P:(jj + 1) * P],
                                 self.oh[:, j * P:(j + 1) * P], rqT[:],
                                 start=True, stop=True)
            nc.vector.tensor_tensor(out=qh[:, hf * CH:(hf + 1) * CH],
                                    in0=qa[:, hf * CH:(hf + 1) * CH],
                                    in1=pq[:], op=OP.mult)
        # k
        k_fm = project(1)
        rk_tm = head_norm(k_fm, 1.0)
        kh = self.wk1.tile([P, T], fr, name=self.name("kh"), tag="kh")
        rope(k_fm, kh)

        # v (+ value embeddings)
        if li in VE_LAYERS:
            vj = VE_LAYERS[li]
            ve_tm = self.wk1.tile([P, NT, P], f32, name=self.name("ve_tm"), tag="tsw")
            for j in range(NT):
                nc.gpsimd.indirect_dma_start(
                    out=ve_tm[:, j, :], out_offset=None, in_=ia[f"ve{vj}"][:],
                    in_offset=IndirectOffsetOnAxis(ap=self.idx_all[:, j:j + 1], axis=0),
                )
            gate_row = self.wk1.tile([1, T], f32, name=self.name("gate_row"), tag="row")
            vegw = self.wt1.tile([P, 1], fr, name=self.name("vegwb"), tag="wblk1")
            nc.sync.dma_start(vegw[:], ia["vegw"][vj])
            for ch in range(NCH):
                pg = self.ps.tile([P, CH], f32, name=self.name("ps_vg"), tag="ps")
                nc.tensor.matmul(pg[0:1, :], vegw[:],
                                 xn_fm[:, 0, ch * CH:(ch + 1) * CH],
                                 start=True, stop=True)
                nc.scalar.activation(gate_row[:, ch * CH:(ch + 1) * CH], pg[0:1, :],
                                     AF.Sigmoid)
            nc.vector.tensor_scalar(out=gate_row[:], in0=gate_row[:], scalar1=2.0,
                                    scalar2=None, op0=OP.mult)
            gate_tm = self.row_to_tm(gate_row)
        else:
            ve_tm, gate_tm = None, None

        v_fm = project(2)
        v_tm = self.wk1.tile([P, NT, P], fr, name=self.name("v_tm"), tag="vtm")
        for j in range(NT):
            pt = self.ps.tile([P, CH], f32, name=self.name("ps_vt"), tag="ps")
            nc.tensor.transpose(pt[:, :P], v_fm[:, j * P:(j + 1) * P], self.idn[:])
            if ve_tm is None:
                self.cp(j, v_tm[:, j, :], pt[:, :P])
            else:
                nc.vector.scalar_tensor_tensor(
                    out=v_tm[:, j, :], in0=ve_tm[:, j, :], scalar=gate_tm[:, j:j + 1],
                    in1=pt[:, :P], op0=OP.mult, op1=OP.add)

        # scores -> exp -> (den, pv) interleaved; p_j streamed per (ch, j)
        den_row = self.wk1.tile([1, T], f32, name=self.name("den_row"), tag="row")
        y_sb = self.wk1.tile([P, T], fr, name=self.name("y_sb"), tag="qkv")
        for ch in range(NCH):
            live = [j for j in range(NT) if plan[(j, ch)] != "skip"]
            # PSUM ring is 8 slots; pd+py persist, so <=6 pm tiles per group
            groups = [live[i:i + 6] for i in range(0, len(live), 6)]
            ds = den_row[:, ch * CH:(ch + 1) * CH]
            ys = y_sb[:, ch * CH:(ch + 1) * CH]
            for gi, grp in enumerate(groups):
                pd = self.ps.tile([P, CH], f32, name=self.name("ps_den"), tag="ps")
                py = self.ps.tile([P, CH], f32, name=self.name("ps_pv"), tag="ps")
                for n, j in enumerate(grp):
                    kind = plan[(j, ch)]
                    pm = self.ps.tile([P, CH], f32, name=self.name("ps_sc"), tag="ps")
                    nc.tensor.matmul(pm[:], kh[:, j * P:(j + 1) * P],
                                     qh[:, ch * CH:(ch + 1) * CH],
                                     start=True, stop=True)
                    pj = wk.tile([P, CH], fr, name=self.name("p_j"), tag="pj")
                    nc.scalar.activation(pj[:], pm[:], AF.Exp,
                                         scale=rk_tm[:, j:j + 1],
                                         bias=self.expb[:, :1])
                    if kind != "full":
                        nc.vector.tensor_tensor(out=pj[:], in0=pj[:],
                                                in1=self.masks[:, kind, :],
                                                op=OP.mult)
                    nc.tensor.matmul(pd[0:1, :], self.ones_fr[:], pj[:],
                                     start=(n == 0), stop=(n == len(grp) - 1),
                                     skip_group_check=True)
                    nc.tensor.matmul(py[:], v_tm[:, j, :], pj[:],
                                     start=(n == 0), stop=(n == len(grp) - 1),
                                     skip_group_check=True)
                if gi == 0:
                    nc.scalar.copy(ds, pd[0:1, :])
                    self.cp(ch, ys, py[:])
                else:
                    nc.vector.tensor_add(out=ds, in0=ds, in1=pd[0:1, :])
                    nc.vector.tensor_add(out=ys, in0=ys, in1=py[:])
        den_tm = self.row_to_tm(den_row)
        rden_tm = sm.tile([P, NT], f32, name=self.name("rden"), tag="smn")
        nc.vector.reciprocal(rden_tm[:], den_tm[:])

        # o-proj partial (token-major; 1/den folded) -> rsin -> RS (f32)
        rsin = self.dram("rsin_a", [T, D], f32)
        for tb in range(NT):
            for ch in range(NCH):
                po = self.ps.tile([P, CH], f32, name=self.name("ps_op"), tag="ps")
                nc.tensor.matmul(po[:], y_sb[:, tb * P:(tb + 1) * P],
                                 wo[:, ch * CH:(ch + 1) * CH], start=True, stop=True)
                ot = wk.tile([P, CH], f32, name=self.name("o_stg"), tag="stg32")
                nc.scalar.mul(ot[:], po[:], rden_tm[:, tb:tb + 1])
                nc.sync.dma_start(
                    rsin.ap()[tb * P:(tb + 1) * P, ch * CH:(ch + 1) * CH], ot[:])
        self.reduce_scatter_add(rsin, x, f32)

        # ---- MLP / MoE ------------------------------------------------------
        xm = self.rms_tm(self.wk1, "rmsout", x)
        lowp = li == L - 1
        xm_fm, xmT, xmT32 = self.allgather_fm(
            xm, f"xm_fm{li}", dt=f16 if lowp else fr, want_f32T=lowp and moe_layer)
        rsin2 = self.dram("rsin_m", [T, D], f16 if lowp else f32)
        if not moe_layer:
            self.dense_mlp(li, xm_fm, rsin2)
        else:
            rt = xmT32 if lowp else xmT
            self.moe(li - DENSE_N, xm_fm, rt, rsin2, f16 if lowp else fr)
        self.reduce_scatter_add(rsin2, x, f16 if lowp else f32)

    # ---- dense mlp (F-sharded 512 per core; f32r) ---------------------------
    def dense_mlp(self, li, xm_fm, rsin2):
        nc, ia, wk = self.nc, self.ia, self.wk
        h2 = self.big.tile([P, NF, T], fr, name=self.name("h2"), tag="big")
        for fb in range(NF):
            w1s = self.w1p.tile([P, ND * P], fr, name=self.name("fcs"), tag="w1s")
            nc.sync.dma_start(w1s[:], ia["fc_s"][li, :, fb * D:(fb + 1) * D])
            pms = [self.ps.tile([P, CH], f32, name=self.name("ps_fc"), tag="ps")
                   for _ in range(NCH)]
            for db in range(ND):
                for ch in range(NCH):
                    nc.tensor.matmul(
                        pms[ch][:], w1s[:, db * P:(db + 1) * P],
                        xm_fm[:, db, ch * CH:(ch + 1) * CH],
                        start=(db == 0), stop=(db == ND - 1),
                        skip_group_check=True)
            for ch in range(NCH):
                rl = wk.tile([P, CH], fr, name=self.name("rl"), tag="sg")
                nc.scalar.activation(rl[:], pms[ch][:], AF.Relu)
                nc.vector.tensor_tensor(out=h2[:, fb, ch * CH:(ch + 1) * CH],
                                        in0=rl[:], in1=rl[:], op=OP.mult)
        for ch in range(NCH):
            psums = [self.ps.tile([P, CH], f32, name=self.name("ps_pj"), tag="ps")
                     for _ in range(NT)]
            for fb in range(NF):
                wb = self.wr.tile([P, CH], fr, name=self.name("pj_wb"), tag="wrs")
                nc.sync.dma_start(wb[:], ia["proj_s"][li, :,
                                                      fb * D + ch * CH:fb * D + (ch + 1) * CH])
                for tb in range(NT):
                    nc.tensor.matmul(psums[tb][:], h2[:, fb, tb * P:(tb + 1) * P],
                                     wb[:], start=(fb == 0), stop=(fb == NF - 1))
            for tb in range(NT):
                ot = wk.tile([P, CH], f32, name=self.name("pj_stg"), tag="stg32")
                self.cp(tb, ot[:], psums[tb][:])
                nc.sync.dma_start(
                    rsin2.ap()[tb * P:(tb + 1) * P, ch * CH:(ch + 1) * CH], ot[:])

    # ---- MoE (expert-sharded; dense over all tokens) ------------------------
    def moe(self, mi, xm_fm, xmT, rsin2, mdt):
        nc, ia, wk, sm = self.nc, self.ia, self.wk, self.sm
        # routing in fp32 on my resident tokens, then tiny AG (overlapped)
        rw_sb = sm.tile([P, ND, E], f32, name=self.name("rw_sb"), tag="rw")
        nc.sync.dma_start(rw_sb[:], ia["rw"][mi].rearrange("(n p) e -> p n e", p=P)[:])
        pr = self.ps.tile([P, CH], f32, name=self.name("ps_rt"), tag="ps")
        for db in range(ND):
            lh = xmT[:, db, :]
            if lh.dtype != f32:
                lh = lh.bitcast(f32)
            nc.tensor.matmul(pr[:, :E], lh, rw_sb[:, db, :],
                             start=(db == 0), stop=(db == ND - 1))
        nmax = sm.tile([P, 1], f32, name=self.name("nmax"), tag="sm1")
        nc.vector.tensor_reduce(nmax[:], pr[:, :E], axis=AX.X, op=OP.max, negate=True)
        probs = sm.tile([P, E], f32, name=self.name("probs"), tag="smn")
        se = sm.tile([P, 1], f32, name=self.name("se"), tag="sm1")
        nc.scalar.activation(probs[:], pr[:, :E], AF.Exp, bias=nmax[:, :1],
                             accum_out=se[:, :1])
        rse = sm.tile([P, 1], f32, name=self.name("rse"), tag="sm1")
        nc.vector.reciprocal(rse[:], se[:])
        nc.vector.tensor_scalar(out=probs[:], in0=probs[:], scalar1=rse[:, :1],
                                scalar2=None, op0=OP.mult)
        m8 = sm.tile([P, 8], f32, name=self.name("m8"), tag="smn")
        nc.vector.max(m8[:], probs[:])
        wf_my = sm.tile([P, E], f32, name=self.name("wfmy"), tag="smn")
        nc.vector.tensor_scalar(out=wf_my[:], in0=probs[:], scalar1=m8[:, 1:2],
                                scalar2=None, op0=OP.is_ge)
        nc.vector.tensor_tensor(out=wf_my[:], in0=wf_my[:], in1=probs[:], op=OP.mult)
        wfin = self.dram("wfin", [TS, E])
        nc.sync.dma_start(wfin.ap()[:], wf_my[:])
        wfout = self.dram("wfout", [T, E], shared=True)
        nc.gpsimd.collective_compute(
            "AllGather", OP.bypass, replica_groups=[list(range(NCORE))],
            ins=[wfin.ap()[:]], outs=[wfout.ap()[:]],
        )
        wf_all = sm.tile([P, NT, E], f32, name=self.name("wfall"), tag="wfall")
        nc.sync.dma_start(wf_all[:], wfout.ap().rearrange("(j p) e -> p j e", p=P)[:])
        wcol = sm.tile([P, NT], f32, name=self.name("wcol"), tag="wcol")
        wfsel = sm.tile([P, NT, E], f32, name=self.name("wfsel"), tag="wfall")
        nc.vector.tensor_tensor(out=wfsel[:], in0=wf_all[:],
                                in1=self.wsel[:, None, :].to_broadcast([P, NT, E]),
                                op=OP.mult)
        nc.vector.tensor_reduce(wcol[:], wfsel[:], axis=AX.X, op=OP.add)

        gut = self.wt1.tile([P, ND * 2 * P], mdt, name=self.name("gut"), tag="gut")
        nc.sync.dma_start(gut[:], ia[f"gu_s_{mi}"][:])
        down = self.wt1.tile([P, D], mdt, name=self.name("down_sb"), tag="wo")
        nc.sync.dma_start(down[:], ia[f"down_s_{mi}"][:])
        gwt = self.wt1.tile([P, ND], mdt, name=self.name("gwt"), tag="gwt")
        nc.sync.dma_start(gwt[:], ia[f"gatew_{mi}"][:])
        onesrow = self.onesrow16 if mdt == f16 else self.onesrow_fr

        # shared expert: su = sig(gate) * silu(g) * u
        gt_row = self.wk1.tile([1, T], mdt, name=self.name("gt_row"), tag="row")
        for ch in range(NCH):
            pg = self.ps.tile([P, CH], f32, name=self.name("ps_sg"), tag="ps")
            for db in range(ND):
                nc.tensor.matmul(pg[0:1, :], gwt[:, db:db + 1],
                                 xm_fm[:, db, ch * CH:(ch + 1) * CH],
                                 start=(db == 0), stop=(db == ND - 1))
            nc.scalar.activation(gt_row[:, ch * CH:(ch + 1) * CH], pg[0:1, :],
                                 AF.Sigmoid)
        su = self.wk1.tile([P, T], mdt, name=self.name("su_sb"), tag="su")
        for ch in range(NCH):
            bc = self.ps.tile([P, CH], f32, name=self.name("ps_bc"), tag="ps")
            nc.tensor.matmul(bc[:], onesrow[:],
                             gt_row[:, ch * CH:(ch + 1) * CH], start=True, stop=True)
            pm_g = self.ps.tile([P, CH], f32, name=self.name("ps_g"), tag="ps")
            for db in range(ND):
                nc.tensor.matmul(pm_g[:], gut[:, db * 2 * P:db * 2 * P + P],
                                 xm_fm[:, db, ch * CH:(ch + 1) * CH],
                                 start=(db == 0), stop=(db == ND - 1))
            pm_u = self.ps.tile([P, CH], f32, name=self.name("ps_u"), tag="ps")
            for db in range(ND):
                nc.tensor.matmul(pm_u[:], gut[:, db * 2 * P + P:(db + 1) * 2 * P],
                                 xm_fm[:, db, ch * CH:(ch + 1) * CH],
                                 start=(db == 0), stop=(db == ND - 1))
            cs = slice(ch * CH, (ch + 1) * CH)
            sg = wk.tile([P, CH], mdt, name=self.name("sg_stg"), tag="sg")
            nc.scalar.activation(sg[:], pm_g[:], AF.Silu)
            nc.vector.tensor_tensor(out=sg[:], in0=sg[:], in1=pm_u[:], op=OP.mult)
            nc.vector.tensor_tensor(out=su[:, cs], in0=sg[:], in1=bc[:], op=OP.mult)

        # routed expert: h = silu(xm @ w1)
        h = self.big.tile([P, ND, T], mdt, name=self.name("h_moe"), tag="big")
        for fb in range(ND):
            w1s = self.w1p.tile([P, ND * P], mdt, name=self.name("w1s"), tag="w1s")
            nc.sync.dma_start(w1s[:], ia[f"w1_{mi}"][:, fb * D:(fb + 1) * D])
            pms = [self.ps.tile([P, CH], f32, name=self.name("ps_w1"), tag="ps")
                   for _ in range(NCH)]
            for db in range(ND):
                for ch in range(NCH):
                    nc.tensor.matmul(pms[ch][:], w1s[:, db * P:(db + 1) * P],
                                     xm_fm[:, db, ch * CH:(ch + 1) * CH],
                                     start=(db == 0), stop=(db == ND - 1),
                                     skip_group_check=True)
            for ch in range(NCH):
                cs = slice(ch * CH, (ch + 1) * CH)
                nc.scalar.activation(h[:, fb, cs], pms[ch][:], AF.Silu)

        # y = wcol * (h @ w2) + su @ down -> rsin2
        odt = f16 if mdt == f16 else f32
        for ch in range(NCH):
            psums = [self.ps.tile([P, CH], f32, name=self.name("ps_w2"), tag="ps")
                     for _ in range(NT)]
            for fb in range(ND):
                wb = self.wr.tile([P, CH], mdt, name=self.name("w2wb"), tag="wrs")
                nc.sync.dma_start(wb[:], ia[f"w2_{mi}"][:,
                                                        fb * D + ch * CH:fb * D + (ch + 1) * CH])
                for tb in range(NT):
                    nc.tensor.matmul(psums[tb][:], h[:, fb, tb * P:(tb + 1) * P],
                                     wb[:], start=(fb == 0), stop=(fb == ND - 1))
            for tb in range(NT):
                ot = wk.tile([P, CH], odt, name=self.name("moe_stg"),
                             tag="stg" if odt == f16 else "stg32")
                nc.scalar.mul(ot[:], psums[tb][:], wcol[:, tb:tb + 1])
                pd = self.ps.tile([P, CH], f32, name=self.name("ps_dn"), tag="ps")
                nc.tensor.matmul(pd[:], su[:, tb * P:(tb + 1) * P],
                                 down[:, ch * CH:(ch + 1) * CH], start=True, stop=True)
                nc.vector.tensor_add(out=ot[:], in0=ot[:], in1=pd[:])
                nc.sync.dma_start(
                    rsin2.ap()[tb * P:(tb + 1) * P, ch * CH:(ch + 1) * CH], ot[:])


# ---------------------------------------------------------------- build + run
_BUILT = None


def _build():
    global _BUILT
    if _BUILT is not None:
        return _BUILT
    nc = bacc.Bacc("TRN2", target_bir_lowering=False, debug=False, num_devices=NCORE)

    def inp(name, shape, dtype=f32):
        return nc.dram_tensor(name, list(shape), dtype, kind="ExternalInput").ap()

    ia = {
        "idx_my": inp("idx_my", [P, 1], i32),
        "idx_all": inp("idx_all", [P, NT], i32),
        "wte": inp("wte", [V, D]),
        "ve0": inp("ve0", [V, P]),
        "ve1": inp("ve1", [V, P]),
        "vegw": inp("vegw", [2, P, 1], fr),
        "ones_fr": inp("ones_fr", [P, 1], fr),
        "ones_row_fr": inp("ones_row_fr", [1, P], fr),
        "wqkv": inp("wqkv", [L, P, 3 * D], fr),
        "wo": inp("wo", [L, P, D], fr),
        "fc_s": inp("fc_s", [DENSE_N, P, NF * D], fr),
        "proj_s": inp("proj_s", [DENSE_N, P, NF * D], fr),
        "rw": inp("rw", [2, D, E]),
        "wsel": inp("wsel", [P, E]),
        "w1_0": inp("w1_0", [P, ND * D], fr),
        "w1_1": inp("w1_1", [P, ND * D], f16),
        "w2_0": inp("w2_0", [P, ND * D], fr),
        "w2_1": inp("w2_1", [P, ND * D], f16),
        "gu_s_0": inp("gu_s_0", [P, ND * 2 * P], fr),
        "gu_s_1": inp("gu_s_1", [P, ND * 2 * P], f16),
        "down_s_0": inp("down_s_0", [P, D], fr),
        "down_s_1": inp("down_s_1", [P, D], f16),
        "gatew_0": inp("gatew_0", [P, ND], fr),
        "gatew_1": inp("gatew_1", [P, ND], f16),
        "lmh": inp("lmh", [P, ((VS + P - 1) // P) * ND * P], f16),
        "lam_r": inp("lam_r", [P, L]),
        "lam_x": inp("lam_x", [P, L]),
        "cc": inp("cc", [P, T]),
        "ss": inp("ss", [P, T]),
        "masks": inp("masks", [NMASK, P, CH], f16),
        "oh": inp("oh", [NT, NT * P]),
        "out": nc.dram_tensor("out", [VS, T], f16, kind="ExternalOutput").ap(),
    }
    with tile.TileContext(nc) as tc:
        Builder(nc, tc, ia).build()
    nc.compile()
    _BUILT = nc
    return nc


def _h(a):
    return np.ascontiguousarray(np.asarray(a)).astype(NPH)


def _f(a):
    return np.ascontiguousarray(np.asarray(a), dtype=np.float32)


def make_in_maps(inputs):
    idx = np.asarray(inputs["idx"]).reshape(T).astype(np.int32)
    cc, ss = _rope_tables()
    oh = np.zeros((NT, NT * P), np.float32)
    for j in range(NT):
        oh[j, j * P:(j + 1) * P] = 1.0
    shared = {
        "idx_all": np.ascontiguousarray(idx.reshape(NT, P).T),
        "wte": _f(inputs["wte"]),
        "rw": _f(inputs["router_w"]),
        "lam_r": np.ascontiguousarray(
            np.broadcast_to(np.asarray(inputs["resid_lambdas"], np.float32), (P, L))),
        "lam_x": np.ascontiguousarray(
            np.broadcast_to(np.asarray(inputs["x0_lambdas"], np.float32), (P, L))),
        "cc": cc,
        "ss": ss,
        "ones_fr": np.ones((P, 1), np.float32),
        "ones_row_fr": np.ones((1, P), np.float32),
        "masks": MASKS.astype(NPH),
        "oh": oh,
    }
    wq = np.asarray(inputs["attn_q"])
    wk_ = np.asarray(inputs["attn_k"])
    wv = np.asarray(inputs["attn_v"])
    gu_full = np.asarray(inputs["shared_gu"])
    in_maps = []
    for c in range(NCORE):
        hs = slice(c * P, (c + 1) * P)
        vegw = np.zeros((2, P, 1), np.float32)
        for j in range(2):
            vegw[j, :VE_GATE_CH, 0] = np.asarray(inputs["ve_gate_w"])[j][:, c]
        wqkv = np.empty((L, P, 3 * D), np.float32)
        for l in range(L):
            wqkv[l, :, 0 * D:1 * D] = _pack(_f(wq[l][:, hs]), P)
            wqkv[l, :, 1 * D:2 * D] = _pack(_f(wk_[l][:, hs]), P)
            wqkv[l, :, 2 * D:3 * D] = _pack(_f(wv[l][:, hs]), P)
        fc_s = np.stack([
            _pack_fb(_f(np.asarray(inputs["mlp_fc"])[l][:, c * 512:(c + 1) * 512]), NF)
            for l in range(DENSE_N)])
        proj_s = np.stack([
            _pack(_f(np.asarray(inputs["mlp_proj"])[l][c * 512:(c + 1) * 512, :]), P)
            for l in range(DENSE_N)])
        gu_c = np.empty((2, D, 2 * P), np.float32)
        for m in range(2):
            gu_c[m, :, :P] = gu_full[m][:, c * P:(c + 1) * P]
            gu_c[m, :, P:] = gu_full[m][:, F + c * P:F + (c + 1) * P]
        wsel = np.zeros((P, E), np.float32)
        wsel[:, c] = 1.0
        m = dict(shared)
        m.update({
            "idx_my": np.ascontiguousarray(idx[c * P:(c + 1) * P, None]),
            "ve0": _f(np.asarray(inputs["ve_tables"])[0][:, hs]),
            "ve1": _f(np.asarray(inputs["ve_tables"])[1][:, hs]),
            "vegw": vegw,
            "wqkv": wqkv,
            "wo": _f(np.asarray(inputs["attn_o"])[:, hs, :]),
            "fc_s": fc_s,
            "proj_s": proj_s,
            "wsel": wsel,
            "w1_0": _pack_fb(_f(np.asarray(inputs["moe_w1"])[0, c]), ND),
            "w1_1": _pack_fb(_h(np.asarray(inputs["moe_w1"])[1, c]), ND),
            "w2_0": _pack(_f(np.asarray(inputs["moe_w2"])[0, c]), P),
            "w2_1": _pack(_h(np.asarray(inputs["moe_w2"])[1, c]), P),
            "gu_s_0": _pack(_f(gu_c[0]), P),
            "gu_s_1": _pack(_h(gu_c[1]), P),
            "down_s_0": _f(np.asarray(inputs["shared_down"])[0, c * P:(c + 1) * P, :]),
            "down_s_1": _h(np.asarray(inputs["shared_down"])[1, c * P:(c + 1) * P, :]),
            "gatew_0": _f(np.asarray(inputs["shared_gate_w"])[0].reshape(ND, P).T),
            "gatew_1": _h(np.asarray(inputs["shared_gate_w"])[1].reshape(ND, P).T),
            "lmh": _pack_lmh(_h(np.asarray(inputs["lm_head_w"])[:, c * VS:(c + 1) * VS])),
        })
        in_maps.append(m)
    return in_maps


def kernel(**inputs):
    nc = _build()
    in_maps = make_in_maps(inputs)
    res = run_bass_kernel_spmd(nc, in_maps, list(range(NCORE)))
    outs = [res.results[c]["out"].astype(np.float32).T for c in range(NCORE)]
    return np.concatenate(outs, axis=1).reshape(B, T, V)


if __name__ == "__main__":
    nc = _build()
    n_inst = sum(len(bb.instructions) for bb in nc.main_func.blocks)
    print("build OK; instructions:", n_inst)

